# revision 35
# baseline (speedup 1.0000x reference)
"""MoE hard-routing kernel for Trainium2 (8 NeuronCores, Bass/Tile).

Problem: out[t] = x[t] @ W[p[t]].T + b[p[t]]
  x [8, 4096, 512] f32, partitions [8, 4096] int32 (values 0..7),
  W [8, 512, 512] f32, b [8, 512] f32.

Strategy: expert-parallel sharding. n_experts == n_cores == 8, so core e
owns expert e. The host routes each token to its expert's core (that IS the
shard assignment — a partition of the token set), pre-transposed so d_in
lies on SBUF partitions, and pre-cast to fp16 (tolerance is 2e-2; fp16
operands keep the GEMM rel-err ~1e-3 while halving DMA traffic and keeping
the PE at its full 1 row/cycle rate). Each core runs one dense GEMM
  out_e[d_out, tok] = W[e] @ xT_e  (+ b[e])
accumulated over 4 K-chunks of 128 in fp32 PSUM, with the bias added and
the fp32->fp16 downcast fused into the PSUM eviction.

Per-trace tuning (measured on HW: PE issues one 512-col matmul every
216ns at 2.4 GHz when, and only when, it is never allowed to stall — any
idle gap parks the DVFS at ~1.2-2.0 GHz for several microseconds):
  - x streams as per-K-chunk 128KB transfers on the in-order SP ring: the
    k'th matmul group waits only on its own chunk and the single queue
    self-paces, so the 8 cores' simultaneous bootstrap never splits HBM
    bandwidth across half-loaded superblocks.
  - W rides the ACT ring per K-chunk; the warm-up memset and the first
    W/x triggers are hoisted ahead of the TileContext entry barrier
    (engine streams are in-order and they carry no waits) so their
    transfers overlap the barrier dance.
  - A few warm-up matmuls on zeros bridge the PE from the barrier to the
    first x chunk's arrival.
  - The ACT activation table is preloaded via a dummy Identity activation
    so the first eviction doesn't eat the lazy 1.3us ACT_TABLE_LOAD.
  - PSUM evictions alternate DVE (even m) / ACT (odd m); stores are one
    DMA per 512-column block (HWDGE triggers cost a fixed ~630ns of queue
    time each), with the last two blocks' stores split across both HWDGE
    rings and a short 128-column remainder block so the tail drains fast.
"""

import sys

for _p in ("/opt/trn_rl_repo", "/root/.axon_site/_ro/trn_rl_repo"):
    if _p not in sys.path:
        sys.path.append(_p)

import numpy as np

import concourse.bass as bass
import concourse.mybir as mybir
import concourse.tile as tile
from concourse.bass import ts
from concourse.bass_utils import run_bass_kernel_spmd
import bass_rust as _br

D_IN = 512
D_OUT = 512
N_EXPERTS = 8
N_CORES = 8
P = 128
NBLK = 512  # token columns per matmul (one PSUM bank of fp32)
SBLK = 1024  # token columns per x-load superblock
KC = D_IN // P  # 4 contraction chunks
MC = D_OUT // P  # 4 output-row chunks

MATH_MODE = "f16"

N_WARMUP = 3  # 512-col warm-up matmuls to ramp the PE clock during DMA


def _np_dt(math_mode: str):
    if math_mode == "f16":
        return np.float16
    if math_mode == "bf16":
        import ml_dtypes

        return ml_dtypes.bfloat16
    if math_mode in ("f32r", "f32"):
        return np.float32
    raise ValueError(math_mode)


def _mm_dt(math_mode: str):
    return {
        "f16": mybir.dt.float16,
        "bf16": mybir.dt.bfloat16,
        "f32r": mybir.dt.float32r,
        "f32": mybir.dt.float32,
    }[math_mode]


def _split_multiwait(nc: bass.Bass) -> None:
    """Hoist extra sem waits onto injected same-engine nops.

    The walrus build in this container rejects more than one sync-wait
    command on a single instruction.  Engine queues are in-order, so a
    nop carrying one wait immediately before the instruction is
    semantically identical to the wait being attached directly.
    """
    cnt = 0
    for bb in nc.main_func.blocks:
        new = []
        changed = False
        for ins in bb.instructions:
            si = ins.sync_info
            if si is not None and len(si.on_wait) > 1:
                waits = list(si.on_wait)
                for w in waits[:-1]:
                    nop = mybir.InstNoOp(name=f"wsplit-{cnt}", ins=[], outs=[])
                    cnt += 1
                    nop.engine = ins.engine
                    nop.sync_info = _br.SyncInfo(on_wait=[w], on_update=[])
                    new.append(nop)
                ins.sync_info = _br.SyncInfo(
                    on_wait=[waits[-1]], on_update=list(si.on_update)
                )
                changed = True
            new.append(ins)
        if changed:
            bb.instructions = new


def _hoist_prologue(nc: bass.Bass, quota: dict) -> None:
    """Move the leading wait-free kernel instructions (warm-up memset, first
    W/x DMA triggers) of each engine ahead of the TileContext entry barrier.

    Engine streams are in-order, so an instruction with no sync waits whose
    operands are untouched by the preamble can run before the barrier; its
    transfers then overlap the barrier dance and the first matmul's data is
    resident ~1us earlier.  `quota` maps EngineType -> how many instructions
    to hoist (kept small: each hoisted trigger delays that engine's barrier
    join by its issue time).
    """
    main_bb, tile_bb = nc.main_func.blocks[0], nc.main_func.blocks[1]
    quota = dict(quota)
    hoisted: dict = {e: [] for e in quota}
    blocked: set = set()
    keep = []
    for ins in tile_bb.instructions:
        e = ins.engine
        si = ins.sync_info
        no_wait = si is None or len(si.on_wait) == 0
        if (
            e in quota
            and quota[e] > 0
            and e not in blocked
            and no_wait
            and type(ins).__name__ in ("InstMemset", "InstDMACopy")
        ):
            hoisted[e].append(ins)
            quota[e] -= 1
            continue
        if e in quota:
            blocked.add(e)
        keep.append(ins)
    tile_bb.instructions = keep
    # Insert each engine's hoisted list right before its barrier Drain (the
    # instruction carrying the barrier-gather sem increment).
    new_main = []
    for ins in main_bb.instructions:
        is_gather_drain = type(ins).__name__ == "InstDrain" and (
            ins.sync_info is None  # Pool: barrier coordinator's drain
            or any(
                "gather" in (u.ant_name or "") for u in ins.sync_info.on_update
            )
        )
        if is_gather_drain and hoisted.get(ins.engine):
            new_main.extend(hoisted.pop(ins.engine))
        new_main.append(ins)
    main_bb.instructions = new_main


def _build_nc(C: int, math_mode: str) -> bass.Bass:
    """One core's program: out[512, C] = wT.T-contract(xT) + bias."""
    f32 = mybir.dt.float32
    nc = bass.Bass("TRN2", target_bir_lowering=False, debug=False, num_devices=N_CORES)

    mm_dt = _mm_dt(math_mode)
    out_dt = f32 if math_mode in ("f32r", "f32") else mm_dt

    xT = nc.declare_dram_parameter("xT", [D_IN, C], mm_dt, isOutput=False)
    wT = nc.declare_dram_parameter("wT", [D_IN, D_OUT], mm_dt, isOutput=False)
    bias = nc.declare_dram_parameter("bias", [D_OUT], f32, isOutput=False)
    out = nc.declare_dram_parameter("out", [D_OUT, C], out_dt, isOutput=True)

    # [p, k, c] views of the DRAM operands (k-chunk on the free dims).
    xT_v = xT.rearrange("(k p) c -> p k c", p=P)
    out_v = out.rearrange("(m p) c -> p m c", p=P)
    wT_v = wT.rearrange("(k p) d -> p k d", p=P)

    # Uniform 512-column superblocks with a short (<512) remainder last.
    # Any PE stall parks the DVFS below 2.4 GHz for several microseconds,
    # so the cadence must be gap-free from the first warm-up on.
    col_blocks = []
    off = 0
    while off < C:
        size = min(NBLK, C - off)
        col_blocks.append((off, size))
        off += size

    with tile.TileContext(nc) as tc:
        with (
            tc.tile_pool(name="wpool", bufs=1) as wpool,
            tc.tile_pool(name="xpool", bufs=10) as xpool,
            tc.tile_pool(name="opool", bufs=3) as opool,
            tc.tile_pool(name="pspool", bufs=8, space="PSUM") as pspool,
        ):
            # Weights: wT[d_in, d_out] -> [128, KC, 512]; chunk (k, m) is the
            # stationary operand [K=128, M=128].  Loaded per k-chunk on the
            # ACT HWDGE ring so the x loads own the SP ring.
            w_t = wpool.tile([P, KC, D_OUT], mm_dt)
            for k in range(KC):
                nc.scalar.dma_start(w_t[:, k, :], wT_v[:, k, :])
            # bias[d_out] -> [128, MC]; column m is the per-partition bias of
            # output-row chunk m.
            b_t = wpool.tile([P, MC], f32)
            nc.scalar.dma_start(b_t[:], bias.rearrange("(m p) -> p m", p=P))

            # Warm-up matmuls on a zeroed tile keep the PE busy while the
            # first x/W chunks are in flight so its clock ramps to 2.4 GHz
            # (any PE idle gap parks the DVFS low for ~5us afterwards).
            warm_x = wpool.tile([P, NBLK], mybir.dt.bfloat16)
            nc.gpsimd.memset(warm_x[:], 0)
            # Preload the ACT activation table (Identity) so the first real
            # eviction doesn't pay the lazy 1.3us table load.
            warm_o = wpool.tile([P, 1], f32)
            nc.scalar.activation(
                warm_o[:],
                warm_x[:, :1],
                mybir.ActivationFunctionType.Identity,
                bias=0.0,
            )
            for wi in range(N_WARMUP):
                ps = pspool.tile([P, NBLK], f32, name=f"ps_w{wi}", tag="ps")
                nc.tensor.matmul(
                    ps[:], warm_x[:, :P], warm_x[:], start=True, stop=True
                )

            n_blocks = len(col_blocks)
            for n, (coff, csz) in enumerate(col_blocks):
                is_last = n == n_blocks - 1
                is_tail = n >= n_blocks - 2
                # Per-K-chunk loads on the in-order SP ring.  Four 128KB
                # transfers per block self-pace against the PE (which eats a
                # K-chunk in ~0.86us): the k'th matmul group only waits on
                # its own chunk, and the single queue serializes transfers so
                # early blocks never fight each other for HBM bandwidth (all
                # 8 cores bootstrap simultaneously — parallel bulk loads
                # split the chip bandwidth and arrive uselessly late).
                x_k = []
                for k in range(KC):
                    xt = xpool.tile([P, NBLK], mm_dt, name=f"x_{n}_{k}", tag="xchunk")
                    nc.sync.dma_start(xt[:, :csz], xT_v[:, k, coff : coff + csz])
                    x_k.append(xt)
                o_t = opool.tile([P, MC, csz], out_dt, name=f"o_{n}", tag="o")
                subs = [(o, min(NBLK, csz - o)) for o in range(0, csz, NBLK)]
                for bi, (boff, bsz) in enumerate(subs):
                    ps_m = [
                        pspool.tile([P, NBLK], f32, name=f"ps_{n}_{bi}_{m}", tag="ps")
                        for m in range(MC)
                    ]
                    for k in range(KC):
                        for m in range(MC):
                            nc.tensor.matmul(
                                ps_m[m][:, :bsz],
                                w_t[:, k, ts(m, P)],
                                x_k[k][:, boff : boff + bsz],
                                start=(k == 0),
                                stop=(k == KC - 1),
                            )
                    for m in range(MC):
                        # Steady state alternates DVE (even m) / ACT (odd m).
                        # The final block pairs DVE=m0,m1 / ACT=m2,m3 instead
                        # so each half-store below waits on only one engine's
                        # eviction chain.
                        on_act = (m >= 2) if is_last else (m % 2 == 1)
                        if on_act:
                            nc.scalar.activation(
                                o_t[:, m, boff : boff + bsz],
                                ps_m[m][:, :bsz],
                                mybir.ActivationFunctionType.Identity,
                                bias=b_t[:, m : m + 1],
                            )
                        else:
                            nc.vector.tensor_scalar_add(
                                o_t[:, m, boff : boff + bsz],
                                ps_m[m][:, :bsz],
                                b_t[:, m : m + 1],
                            )
                # One store DMA per superblock.  The last two blocks split
                # their stores across both HWDGE rings (SP + ACT) so the
                # final transfers drain in parallel and the tail stays short.
                if is_tail:
                    # m0/m1 (DVE-evicted in the last block) ride SP; m2/m3
                    # ride ACT — each half triggers as soon as its own
                    # engine's evictions land.
                    nc.sync.dma_start(
                        out_v[:, :2, coff : coff + csz], o_t[:, :2, :csz]
                    )
                    nc.scalar.dma_start(
                        out_v[:, 2:, coff : coff + csz], o_t[:, 2:, :csz]
                    )
                else:
                    nc.scalar.dma_start(
                        out_v[:, :, coff : coff + csz], o_t[:, :, :csz]
                    )
    _split_multiwait(nc)
    _hoist_prologue(
        nc,
        {
            mybir.EngineType.Pool: 1,  # warm-up memset
            mybir.EngineType.Activation: 2,  # W k0, k1
            mybir.EngineType.SP: 2,  # x block-0 k0, k1
        },
    )
    return nc


_NC_CACHE: dict = {}


def _get_nc(C: int, math_mode: str) -> bass.Bass:
    key = (C, math_mode)
    if key not in _NC_CACHE:
        _NC_CACHE[key] = _build_nc(C, math_mode)
    return _NC_CACHE[key]


def kernel(x: np.ndarray, partitions: np.ndarray, W: np.ndarray, b: np.ndarray,
           _math_mode: str | None = None, _trace: bool = False):
    math_mode = _math_mode or MATH_MODE
    np_dt = _np_dt(math_mode)
    B, S, d_in = x.shape
    n_exp, d_out, _ = W.shape
    assert d_in == D_IN and d_out == D_OUT and n_exp == N_EXPERTS

    xf = np.ascontiguousarray(x, dtype=np.float32).reshape(-1, d_in)
    p = partitions.reshape(-1)

    tok_ids = [np.nonzero(p == e)[0] for e in range(N_EXPERTS)]
    max_cnt = max(len(ids) for ids in tok_ids)
    C = max(NBLK, ((max_cnt + P - 1) // P) * P)

    in_maps = []
    for e in range(N_EXPERTS):
        ids = tok_ids[e]
        xT = np.zeros((D_IN, C), np_dt)
        xT[:, : len(ids)] = xf[ids].T.astype(np_dt)
        in_maps.append(
            {
                "xT": xT,
                "wT": np.ascontiguousarray(W[e].T).astype(np_dt),
                "bias": np.ascontiguousarray(b[e], dtype=np.float32),
            }
        )

    nc = _get_nc(C, math_mode)
    res = run_bass_kernel_spmd(nc, in_maps, list(range(N_CORES)), trace=_trace)

    outf = np.empty((B * S, d_out), np.float32)
    for e in range(N_EXPERTS):
        ids = tok_ids[e]
        outf[ids] = np.asarray(res.results[e]["out"])[:, : len(ids)].T.astype(
            np.float32
        )
    out = outf.reshape(B, S, d_out)
    if _trace:
        return out, res
    return out


# revision 36
# speedup vs baseline: 1.0140x; 1.0140x over previous
"""MoE hard-routing kernel for Trainium2 (8 NeuronCores, Bass/Tile).

Problem: out[t] = x[t] @ W[p[t]].T + b[p[t]]
  x [8, 4096, 512] f32, partitions [8, 4096] int32 (values 0..7),
  W [8, 512, 512] f32, b [8, 512] f32.

Strategy: expert-parallel sharding. n_experts == n_cores == 8, so core e
owns expert e. The host routes each token to its expert's core (that IS the
shard assignment — a partition of the token set), pre-transposed so d_in
lies on SBUF partitions, and pre-cast to fp16 (tolerance is 2e-2; fp16
operands keep the GEMM rel-err ~1e-3 while halving DMA traffic and keeping
the PE at its full 1 row/cycle rate). Each core runs one dense GEMM
  out_e[d_out, tok] = W[e] @ xT_e  (+ b[e])
accumulated over 4 K-chunks of 128 in fp32 PSUM, with the bias added and
the fp32->fp16 downcast fused into the PSUM eviction.

Per-trace tuning (measured on HW: PE issues one 512-col matmul every
216ns at 2.4 GHz when, and only when, it is never allowed to stall — any
idle gap parks the DVFS at ~1.2-2.0 GHz for several microseconds):
  - x streams as per-K-chunk 128KB transfers on the in-order SP ring: the
    k'th matmul group waits only on its own chunk and the single queue
    self-paces, so the 8 cores' simultaneous bootstrap never splits HBM
    bandwidth across half-loaded superblocks.
  - W rides the ACT ring per K-chunk; the warm-up memset and the first
    W/x triggers are hoisted ahead of the TileContext entry barrier
    (engine streams are in-order and they carry no waits) so their
    transfers overlap the barrier dance.
  - A few warm-up matmuls on zeros bridge the PE from the barrier to the
    first x chunk's arrival.
  - The ACT activation table is preloaded via a dummy Identity activation
    so the first eviction doesn't eat the lazy 1.3us ACT_TABLE_LOAD.
  - PSUM evictions alternate DVE (even m) / ACT (odd m); stores are one
    DMA per 512-column block (HWDGE triggers cost a fixed ~630ns of queue
    time each), with the last two blocks' stores split across both HWDGE
    rings and a short 128-column remainder block so the tail drains fast.
"""

import sys

for _p in ("/opt/trn_rl_repo", "/root/.axon_site/_ro/trn_rl_repo"):
    if _p not in sys.path:
        sys.path.append(_p)

import numpy as np

import concourse.bass as bass
import concourse.mybir as mybir
import concourse.tile as tile
from concourse.bass import ts
from concourse.bass_utils import run_bass_kernel_spmd
import bass_rust as _br

D_IN = 512
D_OUT = 512
N_EXPERTS = 8
N_CORES = 8
P = 128
NBLK = 512  # token columns per matmul (one PSUM bank of fp32)
SBLK = 1024  # token columns per x-load superblock
KC = D_IN // P  # 4 contraction chunks
MC = D_OUT // P  # 4 output-row chunks

MATH_MODE = "f16"

N_WARMUP = 4  # 512-col warm-up matmuls to ramp the PE clock during DMA


def _np_dt(math_mode: str):
    if math_mode == "f16":
        return np.float16
    if math_mode == "bf16":
        import ml_dtypes

        return ml_dtypes.bfloat16
    if math_mode in ("f32r", "f32"):
        return np.float32
    raise ValueError(math_mode)


def _mm_dt(math_mode: str):
    return {
        "f16": mybir.dt.float16,
        "bf16": mybir.dt.bfloat16,
        "f32r": mybir.dt.float32r,
        "f32": mybir.dt.float32,
    }[math_mode]


def _split_multiwait(nc: bass.Bass) -> None:
    """Hoist extra sem waits onto injected same-engine nops.

    The walrus build in this container rejects more than one sync-wait
    command on a single instruction.  Engine queues are in-order, so a
    nop carrying one wait immediately before the instruction is
    semantically identical to the wait being attached directly.
    """
    cnt = 0
    for bb in nc.main_func.blocks:
        new = []
        changed = False
        for ins in bb.instructions:
            si = ins.sync_info
            if si is not None and len(si.on_wait) > 1:
                waits = list(si.on_wait)
                for w in waits[:-1]:
                    nop = mybir.InstNoOp(name=f"wsplit-{cnt}", ins=[], outs=[])
                    cnt += 1
                    nop.engine = ins.engine
                    nop.sync_info = _br.SyncInfo(on_wait=[w], on_update=[])
                    new.append(nop)
                ins.sync_info = _br.SyncInfo(
                    on_wait=[waits[-1]], on_update=list(si.on_update)
                )
                changed = True
            new.append(ins)
        if changed:
            bb.instructions = new


def _hoist_prologue(nc: bass.Bass, quota: dict) -> None:
    """Move the leading wait-free kernel instructions (warm-up memset, first
    W/x DMA triggers) of each engine ahead of the TileContext entry barrier.

    Engine streams are in-order, so an instruction with no sync waits whose
    operands are untouched by the preamble can run before the barrier; its
    transfers then overlap the barrier dance and the first matmul's data is
    resident ~1us earlier.  `quota` maps EngineType -> how many instructions
    to hoist (kept small: each hoisted trigger delays that engine's barrier
    join by its issue time).
    """
    main_bb, tile_bb = nc.main_func.blocks[0], nc.main_func.blocks[1]
    quota = dict(quota)
    hoisted: dict = {e: [] for e in quota}
    blocked: set = set()
    keep = []
    for ins in tile_bb.instructions:
        e = ins.engine
        si = ins.sync_info
        no_wait = si is None or len(si.on_wait) == 0
        if (
            e in quota
            and quota[e] > 0
            and e not in blocked
            and no_wait
            and type(ins).__name__ in ("InstMemset", "InstDMACopy")
        ):
            hoisted[e].append(ins)
            quota[e] -= 1
            continue
        if e in quota:
            blocked.add(e)
        keep.append(ins)
    tile_bb.instructions = keep
    # Insert each engine's hoisted list right before its barrier Drain (the
    # instruction carrying the barrier-gather sem increment).
    new_main = []
    for ins in main_bb.instructions:
        is_gather_drain = type(ins).__name__ == "InstDrain" and (
            ins.sync_info is None  # Pool: barrier coordinator's drain
            or any(
                "gather" in (u.ant_name or "") for u in ins.sync_info.on_update
            )
        )
        if is_gather_drain and hoisted.get(ins.engine):
            new_main.extend(hoisted.pop(ins.engine))
        new_main.append(ins)
    main_bb.instructions = new_main


def _build_nc(C: int, math_mode: str) -> bass.Bass:
    """One core's program: out[512, C] = wT.T-contract(xT) + bias."""
    f32 = mybir.dt.float32
    nc = bass.Bass("TRN2", target_bir_lowering=False, debug=False, num_devices=N_CORES)

    mm_dt = _mm_dt(math_mode)
    out_dt = f32 if math_mode in ("f32r", "f32") else mm_dt

    xT = nc.declare_dram_parameter("xT", [D_IN, C], mm_dt, isOutput=False)
    wT = nc.declare_dram_parameter("wT", [D_IN, D_OUT], mm_dt, isOutput=False)
    bias = nc.declare_dram_parameter("bias", [D_OUT], f32, isOutput=False)
    out = nc.declare_dram_parameter("out", [D_OUT, C], out_dt, isOutput=True)

    # [p, k, c] views of the DRAM operands (k-chunk on the free dims).
    xT_v = xT.rearrange("(k p) c -> p k c", p=P)
    out_v = out.rearrange("(m p) c -> p m c", p=P)
    wT_v = wT.rearrange("(k p) d -> p k d", p=P)

    # Uniform 512-column superblocks with a short (<512) remainder last.
    # Any PE stall parks the DVFS below 2.4 GHz for several microseconds,
    # so the cadence must be gap-free from the first warm-up on.
    col_blocks = []
    off = 0
    while off < C:
        size = min(NBLK, C - off)
        col_blocks.append((off, size))
        off += size

    with tile.TileContext(nc) as tc:
        with (
            tc.tile_pool(name="wpool", bufs=1) as wpool,
            tc.tile_pool(name="xpool", bufs=10) as xpool,
            tc.tile_pool(name="opool", bufs=3) as opool,
            tc.tile_pool(name="pspool", bufs=8, space="PSUM") as pspool,
        ):
            # Weights: wT[d_in, d_out] -> [128, KC, 512]; chunk (k, m) is the
            # stationary operand [K=128, M=128].  Loaded per k-chunk on the
            # ACT HWDGE ring so the x loads own the SP ring.
            w_t = wpool.tile([P, KC, D_OUT], mm_dt)
            for k in range(KC):
                nc.scalar.dma_start(w_t[:, k, :], wT_v[:, k, :])
            # bias[d_out] -> [128, MC]; column m is the per-partition bias of
            # output-row chunk m.
            b_t = wpool.tile([P, MC], f32)
            nc.scalar.dma_start(b_t[:], bias.rearrange("(m p) -> p m", p=P))

            # Warm-up matmuls on a zeroed tile keep the PE busy while the
            # first x/W chunks are in flight so its clock ramps to 2.4 GHz
            # (any PE idle gap parks the DVFS low for ~5us afterwards).
            warm_x = wpool.tile([P, NBLK], mybir.dt.bfloat16)
            nc.gpsimd.memset(warm_x[:], 0)
            # Preload the ACT activation table (Identity) so the first real
            # eviction doesn't pay the lazy 1.3us table load.
            warm_o = wpool.tile([P, 1], f32)
            nc.scalar.activation(
                warm_o[:],
                warm_x[:, :1],
                mybir.ActivationFunctionType.Identity,
                bias=0.0,
            )
            for wi in range(N_WARMUP):
                ps = pspool.tile([P, NBLK], f32, name=f"ps_w{wi}", tag="ps")
                nc.tensor.matmul(
                    ps[:], warm_x[:, :P], warm_x[:], start=True, stop=True
                )

            n_blocks = len(col_blocks)
            for n, (coff, csz) in enumerate(col_blocks):
                is_last = n == n_blocks - 1
                is_tail = n >= n_blocks - 2
                # Per-K-chunk loads on the in-order SP ring.  Four 128KB
                # transfers per block self-pace against the PE (which eats a
                # K-chunk in ~0.86us): the k'th matmul group only waits on
                # its own chunk, and the single queue serializes transfers so
                # early blocks never fight each other for HBM bandwidth (all
                # 8 cores bootstrap simultaneously — parallel bulk loads
                # split the chip bandwidth and arrive uselessly late).
                x_k = []
                for k in range(KC):
                    xt = xpool.tile([P, NBLK], mm_dt, name=f"x_{n}_{k}", tag="xchunk")
                    nc.sync.dma_start(xt[:, :csz], xT_v[:, k, coff : coff + csz])
                    x_k.append(xt)
                o_t = opool.tile([P, MC, csz], out_dt, name=f"o_{n}", tag="o")
                subs = [(o, min(NBLK, csz - o)) for o in range(0, csz, NBLK)]
                for bi, (boff, bsz) in enumerate(subs):
                    ps_m = [
                        pspool.tile([P, NBLK], f32, name=f"ps_{n}_{bi}_{m}", tag="ps")
                        for m in range(MC)
                    ]
                    for k in range(KC):
                        for m in range(MC):
                            nc.tensor.matmul(
                                ps_m[m][:, :bsz],
                                w_t[:, k, ts(m, P)],
                                x_k[k][:, boff : boff + bsz],
                                start=(k == 0),
                                stop=(k == KC - 1),
                            )
                    for m in range(MC):
                        # Steady state alternates DVE (even m) / ACT (odd m).
                        # The final block pairs DVE=m0,m1 / ACT=m2,m3 instead
                        # so each half-store below waits on only one engine's
                        # eviction chain.
                        on_act = (m >= 2) if is_last else (m % 2 == 1)
                        if on_act:
                            nc.scalar.activation(
                                o_t[:, m, boff : boff + bsz],
                                ps_m[m][:, :bsz],
                                mybir.ActivationFunctionType.Identity,
                                bias=b_t[:, m : m + 1],
                            )
                        else:
                            nc.vector.tensor_scalar_add(
                                o_t[:, m, boff : boff + bsz],
                                ps_m[m][:, :bsz],
                                b_t[:, m : m + 1],
                            )
                # One store DMA per superblock.  The last two blocks split
                # their stores across both HWDGE rings (SP + ACT) so the
                # final transfers drain in parallel and the tail stays short.
                if is_tail:
                    # m0/m1 (DVE-evicted in the last block) ride SP; m2/m3
                    # ride ACT — each half triggers as soon as its own
                    # engine's evictions land.
                    nc.sync.dma_start(
                        out_v[:, :2, coff : coff + csz], o_t[:, :2, :csz]
                    )
                    nc.scalar.dma_start(
                        out_v[:, 2:, coff : coff + csz], o_t[:, 2:, :csz]
                    )
                else:
                    nc.scalar.dma_start(
                        out_v[:, :, coff : coff + csz], o_t[:, :, :csz]
                    )
    _split_multiwait(nc)
    _hoist_prologue(
        nc,
        {
            mybir.EngineType.Pool: 1,  # warm-up memset
            mybir.EngineType.Activation: 2,  # W k0, k1
            mybir.EngineType.SP: 2,  # x block-0 k0, k1
        },
    )
    return nc


_NC_CACHE: dict = {}


def _get_nc(C: int, math_mode: str) -> bass.Bass:
    key = (C, math_mode)
    if key not in _NC_CACHE:
        _NC_CACHE[key] = _build_nc(C, math_mode)
    return _NC_CACHE[key]


def kernel(x: np.ndarray, partitions: np.ndarray, W: np.ndarray, b: np.ndarray,
           _math_mode: str | None = None, _trace: bool = False):
    math_mode = _math_mode or MATH_MODE
    np_dt = _np_dt(math_mode)
    B, S, d_in = x.shape
    n_exp, d_out, _ = W.shape
    assert d_in == D_IN and d_out == D_OUT and n_exp == N_EXPERTS

    xf = np.ascontiguousarray(x, dtype=np.float32).reshape(-1, d_in)
    p = partitions.reshape(-1)

    tok_ids = [np.nonzero(p == e)[0] for e in range(N_EXPERTS)]
    max_cnt = max(len(ids) for ids in tok_ids)
    C = max(NBLK, ((max_cnt + P - 1) // P) * P)

    in_maps = []
    for e in range(N_EXPERTS):
        ids = tok_ids[e]
        xT = np.zeros((D_IN, C), np_dt)
        xT[:, : len(ids)] = xf[ids].T.astype(np_dt)
        in_maps.append(
            {
                "xT": xT,
                "wT": np.ascontiguousarray(W[e].T).astype(np_dt),
                "bias": np.ascontiguousarray(b[e], dtype=np.float32),
            }
        )

    nc = _get_nc(C, math_mode)
    res = run_bass_kernel_spmd(nc, in_maps, list(range(N_CORES)), trace=_trace)

    outf = np.empty((B * S, d_out), np.float32)
    for e in range(N_EXPERTS):
        ids = tok_ids[e]
        outf[ids] = np.asarray(res.results[e]["out"])[:, : len(ids)].T.astype(
            np.float32
        )
    out = outf.reshape(B, S, d_out)
    if _trace:
        return out, res
    return out


# revision 37
# speedup vs baseline: 1.0605x; 1.0458x over previous
"""MoE hard-routing kernel for Trainium2 (8 NeuronCores, Bass/Tile).

Problem: out[t] = x[t] @ W[p[t]].T + b[p[t]]
  x [8, 4096, 512] f32, partitions [8, 4096] int32 (values 0..7),
  W [8, 512, 512] f32, b [8, 512] f32.

Strategy: expert-parallel sharding. n_experts == n_cores == 8, so core e
owns expert e. The host routes each token to its expert's core (that IS the
shard assignment — a partition of the token set), pre-transposed so d_in
lies on SBUF partitions, and pre-cast to fp16 (tolerance is 2e-2; fp16
operands keep the GEMM rel-err ~1e-3 while halving DMA traffic and keeping
the PE at its full 1 row/cycle rate). Each core runs one dense GEMM
  out_e[d_out, tok] = W[e] @ xT_e  (+ b[e])
accumulated over 4 K-chunks of 128 in fp32 PSUM, with the bias added and
the fp32->fp16 downcast fused into the PSUM eviction.

Per-trace tuning (measured on HW: PE issues one 512-col matmul every
216ns at 2.4 GHz when, and only when, it is never allowed to stall — any
idle gap parks the DVFS at ~1.2-2.0 GHz for several microseconds):
  - x streams as per-K-chunk 128KB transfers on the in-order SP ring: the
    k'th matmul group waits only on its own chunk and the single queue
    self-paces, so the 8 cores' simultaneous bootstrap never splits HBM
    bandwidth across half-loaded superblocks.
  - W rides the ACT ring per K-chunk; the warm-up memset and the first
    W/x triggers are hoisted ahead of the TileContext entry barrier
    (engine streams are in-order and they carry no waits) so their
    transfers overlap the barrier dance.
  - A few warm-up matmuls on zeros bridge the PE from the barrier to the
    first x chunk's arrival.
  - The ACT activation table is preloaded via a dummy Identity activation
    so the first eviction doesn't eat the lazy 1.3us ACT_TABLE_LOAD.
  - PSUM evictions alternate DVE (even m) / ACT (odd m); stores are one
    DMA per 512-column block (HWDGE triggers cost a fixed ~630ns of queue
    time each), with the last two blocks' stores split across both HWDGE
    rings and a short 128-column remainder block so the tail drains fast.
"""

import sys

for _p in ("/opt/trn_rl_repo", "/root/.axon_site/_ro/trn_rl_repo"):
    if _p not in sys.path:
        sys.path.append(_p)

import numpy as np

import concourse.bass as bass
import concourse.mybir as mybir
import concourse.tile as tile
from concourse.bass import ts
from concourse.bass_utils import run_bass_kernel_spmd
import bass_rust as _br

D_IN = 512
D_OUT = 512
N_EXPERTS = 8
N_CORES = 8
P = 128
NBLK = 512  # token columns per matmul (one PSUM bank of fp32)
SBLK = 1024  # token columns per x-load superblock
KC = D_IN // P  # 4 contraction chunks
MC = D_OUT // P  # 4 output-row chunks

MATH_MODE = "f16"

N_WARMUP = 3  # 512-col warm-up matmuls to ramp the PE clock during DMA


def _np_dt(math_mode: str):
    if math_mode == "f16":
        return np.float16
    if math_mode == "bf16":
        import ml_dtypes

        return ml_dtypes.bfloat16
    if math_mode in ("f32r", "f32"):
        return np.float32
    raise ValueError(math_mode)


def _mm_dt(math_mode: str):
    return {
        "f16": mybir.dt.float16,
        "bf16": mybir.dt.bfloat16,
        "f32r": mybir.dt.float32r,
        "f32": mybir.dt.float32,
    }[math_mode]


def _split_multiwait(nc: bass.Bass) -> None:
    """Hoist extra sem waits onto injected same-engine nops.

    The walrus build in this container rejects more than one sync-wait
    command on a single instruction.  Engine queues are in-order, so a
    nop carrying one wait immediately before the instruction is
    semantically identical to the wait being attached directly.
    """
    cnt = 0
    for bb in nc.main_func.blocks:
        new = []
        changed = False
        for ins in bb.instructions:
            si = ins.sync_info
            if si is not None and len(si.on_wait) > 1:
                waits = list(si.on_wait)
                for w in waits[:-1]:
                    nop = mybir.InstNoOp(name=f"wsplit-{cnt}", ins=[], outs=[])
                    cnt += 1
                    nop.engine = ins.engine
                    nop.sync_info = _br.SyncInfo(on_wait=[w], on_update=[])
                    new.append(nop)
                ins.sync_info = _br.SyncInfo(
                    on_wait=[waits[-1]], on_update=list(si.on_update)
                )
                changed = True
            new.append(ins)
        if changed:
            bb.instructions = new


def _hoist_prologue(nc: bass.Bass, quota: dict) -> None:
    """Move the leading wait-free kernel instructions (warm-up memset, first
    W/x DMA triggers) of each engine ahead of the TileContext entry barrier.

    Engine streams are in-order, so an instruction with no sync waits whose
    operands are untouched by the preamble can run before the barrier; its
    transfers then overlap the barrier dance and the first matmul's data is
    resident ~1us earlier.  `quota` maps EngineType -> how many instructions
    to hoist (kept small: each hoisted trigger delays that engine's barrier
    join by its issue time).
    """
    main_bb, tile_bb = nc.main_func.blocks[0], nc.main_func.blocks[1]
    quota = dict(quota)
    hoisted: dict = {e: [] for e in quota}
    blocked: set = set()
    keep = []
    for ins in tile_bb.instructions:
        e = ins.engine
        si = ins.sync_info
        no_wait = si is None or len(si.on_wait) == 0
        if (
            e in quota
            and quota[e] > 0
            and e not in blocked
            and no_wait
            and type(ins).__name__ in ("InstMemset", "InstDMACopy")
        ):
            hoisted[e].append(ins)
            quota[e] -= 1
            continue
        if e in quota:
            blocked.add(e)
        keep.append(ins)
    tile_bb.instructions = keep
    # Insert each engine's hoisted list right before its barrier Drain (the
    # instruction carrying the barrier-gather sem increment).
    new_main = []
    for ins in main_bb.instructions:
        is_gather_drain = type(ins).__name__ == "InstDrain" and (
            ins.sync_info is None  # Pool: barrier coordinator's drain
            or any(
                "gather" in (u.ant_name or "") for u in ins.sync_info.on_update
            )
        )
        if is_gather_drain and hoisted.get(ins.engine):
            new_main.extend(hoisted.pop(ins.engine))
        new_main.append(ins)
    main_bb.instructions = new_main


def _build_nc(C: int, math_mode: str) -> bass.Bass:
    """One core's program: out[512, C] = wT.T-contract(xT) + bias."""
    f32 = mybir.dt.float32
    nc = bass.Bass("TRN2", target_bir_lowering=False, debug=False, num_devices=N_CORES)

    mm_dt = _mm_dt(math_mode)
    out_dt = f32 if math_mode in ("f32r", "f32") else mm_dt

    xT = nc.declare_dram_parameter("xT", [D_IN, C], mm_dt, isOutput=False)
    wT = nc.declare_dram_parameter("wT", [D_IN, D_OUT], mm_dt, isOutput=False)
    bias = nc.declare_dram_parameter("bias", [D_OUT], f32, isOutput=False)
    out = nc.declare_dram_parameter("out", [D_OUT, C], out_dt, isOutput=True)

    # [p, k, c] views of the DRAM operands (k-chunk on the free dims).
    xT_v = xT.rearrange("(k p) c -> p k c", p=P)
    out_v = out.rearrange("(m p) c -> p m c", p=P)
    wT_v = wT.rearrange("(k p) d -> p k d", p=P)

    # Uniform 512-column superblocks with a short (<512) remainder last.
    # Any PE stall parks the DVFS below 2.4 GHz for several microseconds,
    # so the cadence must be gap-free from the first warm-up on.
    col_blocks = []
    off = 0
    while off < C:
        size = min(NBLK, C - off)
        col_blocks.append((off, size))
        off += size

    with tile.TileContext(nc) as tc:
        with (
            tc.tile_pool(name="wpool", bufs=1) as wpool,
            tc.tile_pool(name="xpool", bufs=10) as xpool,
            tc.tile_pool(name="opool", bufs=3) as opool,
            tc.tile_pool(name="pspool", bufs=8, space="PSUM") as pspool,
        ):
            # Weights: wT[d_in, d_out] -> [128, KC, 512]; chunk (k, m) is the
            # stationary operand [K=128, M=128].  Loaded per k-chunk on the
            # ACT HWDGE ring so the x loads own the SP ring.
            w_t = wpool.tile([P, KC, D_OUT], mm_dt)
            for k in range(KC):
                nc.scalar.dma_start(w_t[:, k, :], wT_v[:, k, :])
            # bias[d_out] -> [128, MC]; column m is the per-partition bias of
            # output-row chunk m.
            b_t = wpool.tile([P, MC], f32)
            nc.scalar.dma_start(b_t[:], bias.rearrange("(m p) -> p m", p=P))

            # Warm-up matmuls on a zeroed tile keep the PE busy while the
            # first x/W chunks are in flight so its clock ramps to 2.4 GHz
            # (any PE idle gap parks the DVFS low for ~5us afterwards).
            warm_x = wpool.tile([P, NBLK], mybir.dt.bfloat16)
            nc.gpsimd.memset(warm_x[:], 0)
            # Preload the ACT activation table (Identity) so the first real
            # eviction doesn't pay the lazy 1.3us table load.
            warm_o = wpool.tile([P, 1], f32)
            nc.scalar.activation(
                warm_o[:],
                warm_x[:, :1],
                mybir.ActivationFunctionType.Identity,
                bias=0.0,
            )
            for wi in range(N_WARMUP):
                ps = pspool.tile([P, NBLK], f32, name=f"ps_w{wi}", tag="ps")
                nc.tensor.matmul(
                    ps[:], warm_x[:, :P], warm_x[:], start=True, stop=True
                )

            n_blocks = len(col_blocks)
            for n, (coff, csz) in enumerate(col_blocks):
                is_last = n == n_blocks - 1
                is_tail = n >= n_blocks - 2
                # Per-K-chunk loads on the in-order SP ring.  Four 128KB
                # transfers per block self-pace against the PE (which eats a
                # K-chunk in ~0.86us): the k'th matmul group only waits on
                # its own chunk, and the single queue serializes transfers so
                # early blocks never fight each other for HBM bandwidth (all
                # 8 cores bootstrap simultaneously — parallel bulk loads
                # split the chip bandwidth and arrive uselessly late).
                x_k = []
                for k in range(KC):
                    xt = xpool.tile([P, NBLK], mm_dt, name=f"x_{n}_{k}", tag="xchunk")
                    nc.sync.dma_start(xt[:, :csz], xT_v[:, k, coff : coff + csz])
                    x_k.append(xt)
                o_t = opool.tile([P, MC, csz], out_dt, name=f"o_{n}", tag="o")
                subs = [(o, min(NBLK, csz - o)) for o in range(0, csz, NBLK)]
                for bi, (boff, bsz) in enumerate(subs):
                    ps_m = [
                        pspool.tile([P, NBLK], f32, name=f"ps_{n}_{bi}_{m}", tag="ps")
                        for m in range(MC)
                    ]
                    for k in range(KC):
                        for m in range(MC):
                            nc.tensor.matmul(
                                ps_m[m][:, :bsz],
                                w_t[:, k, ts(m, P)],
                                x_k[k][:, boff : boff + bsz],
                                start=(k == 0),
                                stop=(k == KC - 1),
                            )
                    for m in range(MC):
                        # Steady state alternates DVE (even m) / ACT (odd m).
                        # The final block pairs DVE=m0,m1 / ACT=m2,m3 instead
                        # so each half-store below waits on only one engine's
                        # eviction chain.
                        on_act = (m >= 2) if is_last else (m % 2 == 1)
                        if on_act:
                            nc.scalar.activation(
                                o_t[:, m, boff : boff + bsz],
                                ps_m[m][:, :bsz],
                                mybir.ActivationFunctionType.Identity,
                                bias=b_t[:, m : m + 1],
                            )
                        else:
                            nc.vector.tensor_scalar_add(
                                o_t[:, m, boff : boff + bsz],
                                ps_m[m][:, :bsz],
                                b_t[:, m : m + 1],
                            )
                # One store DMA per superblock.  The last two blocks split
                # their stores across both HWDGE rings (SP + ACT) so the
                # final transfers drain in parallel and the tail stays short.
                if is_tail:
                    # m0/m1 (DVE-evicted in the last block) ride SP; m2/m3
                    # ride ACT — each half triggers as soon as its own
                    # engine's evictions land.
                    nc.sync.dma_start(
                        out_v[:, :2, coff : coff + csz], o_t[:, :2, :csz]
                    )
                    nc.scalar.dma_start(
                        out_v[:, 2:, coff : coff + csz], o_t[:, 2:, :csz]
                    )
                else:
                    nc.scalar.dma_start(
                        out_v[:, :, coff : coff + csz], o_t[:, :, :csz]
                    )
    _split_multiwait(nc)
    _hoist_prologue(
        nc,
        {
            mybir.EngineType.Pool: 1,  # warm-up memset
            mybir.EngineType.Activation: 2,  # W k0, k1
            mybir.EngineType.SP: 2,  # x block-0 k0, k1
        },
    )
    return nc


_NC_CACHE: dict = {}


def _get_nc(C: int, math_mode: str) -> bass.Bass:
    key = (C, math_mode)
    if key not in _NC_CACHE:
        _NC_CACHE[key] = _build_nc(C, math_mode)
    return _NC_CACHE[key]


def kernel(x: np.ndarray, partitions: np.ndarray, W: np.ndarray, b: np.ndarray,
           _math_mode: str | None = None, _trace: bool = False):
    math_mode = _math_mode or MATH_MODE
    np_dt = _np_dt(math_mode)
    B, S, d_in = x.shape
    n_exp, d_out, _ = W.shape
    assert d_in == D_IN and d_out == D_OUT and n_exp == N_EXPERTS

    xf = np.ascontiguousarray(x, dtype=np.float32).reshape(-1, d_in)
    p = partitions.reshape(-1)

    tok_ids = [np.nonzero(p == e)[0] for e in range(N_EXPERTS)]
    max_cnt = max(len(ids) for ids in tok_ids)
    C = max(NBLK, ((max_cnt + P - 1) // P) * P)

    in_maps = []
    for e in range(N_EXPERTS):
        ids = tok_ids[e]
        xT = np.zeros((D_IN, C), np_dt)
        xT[:, : len(ids)] = xf[ids].T.astype(np_dt)
        in_maps.append(
            {
                "xT": xT,
                "wT": np.ascontiguousarray(W[e].T).astype(np_dt),
                "bias": np.ascontiguousarray(b[e], dtype=np.float32),
            }
        )

    nc = _get_nc(C, math_mode)
    res = run_bass_kernel_spmd(nc, in_maps, list(range(N_CORES)), trace=_trace)

    outf = np.empty((B * S, d_out), np.float32)
    for e in range(N_EXPERTS):
        ids = tok_ids[e]
        outf[ids] = np.asarray(res.results[e]["out"])[:, : len(ids)].T.astype(
            np.float32
        )
    out = outf.reshape(B, S, d_out)
    if _trace:
        return out, res
    return out
